# revision 25
# baseline (speedup 1.0000x reference)
"""BestRQ loss kernel for 8 Trainium2 NeuronCores (v5).

Math (exact reformulations of the reference):
  - Only masked tokens contribute; their logits row is shared:
    loss = S0 - mean_t L0[target_t], L0 = mask_emb @ W, S0 = logsumexp(L0).
  - target_t = argmax_n score_tn, score_tn = proj_t.emb_n - 0.5|emb_n|^2.
  - S0, L0, qrow (= -0.5|emb|^2 + (beta@P).emb) depend only on frozen
    weights -> host prep (same class as qrow/perm prep in the baseline).
  - Device per 1024-code block: pt17 = beta*score (fp8 matmul, K=17),
    pt18 = beta*score + 20*L0 (K=18, same stationary tile, L0 row last).
    nb = -max(pt17) on DVE; vsum = sum exp(pt18+nb) on ACT.  The winner
    term is exp(20*L0[argmax]); the host combines blocks/cores with a
    logsumexp and divides by 20.
  - beta is folded into the fp8 operand scales (emb x32, lhs x32, qrow x8
    coef 128, L0 x64 coef 0.3125 -> eff beta = 1024, eff beta*delta = 20.0
    exactly; TRN fp8e4 is OCP e4m3, max normal 240).  fp8 per-element
    noise only perturbs near-ties, which is L0-unbiased.
  - 4096 masked tokens -> 4 tiles x 128 per core; <=128 leftovers are
    replicated on every core as a tail tile where each core scores its own
    1/8 of the (block-rotated) codebook; host combines the partials.
"""

import math

import numpy as np

try:
    import concourse.bass as bass  # noqa: F401
except ImportError:  # pragma: no cover
    import sys

    sys.path.insert(0, "/opt/trn_rl_repo")
    import concourse.bass as bass  # noqa: F401

import concourse.mybir as mybir
from concourse import bacc, bass_utils
from concourse.tile import TileContext

F32 = mybir.dt.float32
BF16 = mybir.dt.bfloat16
FP8 = mybir.dt.float8e4

B, T, D, E, N = 16, 512, 256, 16, 8192
NCORES = 8
EPS = 1e-5
NBLK = 8
BLK = N // NBLK

# TRN fp8e4 is OCP e4m3 (max normal 240, has inf) -> keep operands < 240
S_EM = 32.0     # emb row scale
S_LH = 32.0     # proj lhs scale (eff beta = S_EM*S_LH = 1024)
S_Q = 8.0       # qrow row scale
C_Q = 128.0     # qrow lhs coef (C_Q*S_Q = 1024)
S_L = 64.0      # L0 row scale
C_L = 0.3125    # L0 lhs coef -> eff beta*delta = S_L*C_L = 20.0 (exact fp8)
BD = S_L * C_L  # 20.0

_CACHE = {}


def _build_bass(NT, use_tail):
    nc = bacc.Bacc(
        "TRN2", target_bir_lowering=False, debug=False, num_devices=NCORES
    )
    NLN = NT + (1 if use_tail else 0)
    NOUT = NT * 16 + 2
    xsm = nc.dram_tensor("xsm", [128, NT, D], F32, kind="ExternalInput")
    xst = nc.dram_tensor("xst", [128, D], F32, kind="ExternalInput")
    em9 = nc.dram_tensor("em9", [18, N], FP8, kind="ExternalInput")
    ppb = nc.dram_tensor("ppb", [128, 2, E], BF16, kind="ExternalInput")
    lhc = nc.dram_tensor("lhc", [2, 128], FP8, kind="ExternalInput")
    out = nc.dram_tensor("out", [128, NOUT], F32, kind="ExternalOutput")

    AX = mybir.AxisListType.X
    OP = mybir.AluOpType
    AF = mybir.ActivationFunctionType

    with TileContext(nc) as tc:
        with (
            tc.tile_pool(name="cst", bufs=1) as cst,
            tc.tile_pool(name="lnp", bufs=2) as lnp,
            tc.tile_pool(name="ztp", bufs=2) as ztp,
            tc.tile_pool(name="lhp", bufs=1) as lhp,
            tc.tile_pool(name="ovp", bufs=2) as ovp,
            tc.tile_pool(name="p17", bufs=2, space="PSUM") as p17,
            tc.tile_pool(name="p18", bufs=2, space="PSUM") as p18,
        ):
            # ---------------- DMAs (SP + ACT hwdge queues) ----------------
            x0t = cst.tile([128, D], F32)
            xall = cst.tile([128, max(NT - 1, 1), D], F32)
            xtail = cst.tile([128, D], F32)
            em = cst.tile([18, N], FP8)
            pp = cst.tile([128, 2, E], BF16)

            nc.sync.dma_start(x0t[:], xsm[:, 0, :])
            for q in range(4):
                cs = slice(q * (N // 4), (q + 1) * (N // 4))
                nc.sync.dma_start(em[:, cs], em9[:, cs])
            lhsbufs = []
            for i in range(NLN):
                lb = lhp.tile([18, 128], FP8, tag="lhs", name="lhs",
                              bufs=NLN)
                nc.scalar.dma_start(lb[16:18, :], lhc[:, :])
                lhsbufs.append(lb)
            nc.scalar.dma_start(pp[:], ppb[:, :, :])

            # preload the Exp table while DMAs are in flight
            wz = cst.tile([1, 1], F32)
            nc.vector.memset(wz[:], 0.0)
            we = cst.tile([1, 1], F32)
            nc.scalar.activation(we[0:1, :], wz[0:1, :], AF.Exp)

            mv0 = cst.tile([128, 2], F32)
            rstd0 = cst.tile([128, 1], F32)
            mvall = cst.tile([128, 2 * NLN], F32)
            rstd_all = cst.tile([128, NLN], F32)
            etr = cst.tile([128, BLK], BF16)   # exp trash output

            def xin(i):
                if i == 0:
                    return x0t[:]
                return xall[:, i - 1, :] if i < NT else xtail[:]

            # ------------- LN stats; rsqrt via Newton (no tables) ---------
            def ln_stats(i, mv):
                st6 = lnp.tile([128, 6], F32, tag="st6")
                nc.vector.bn_stats(st6[:], xin(i))
                nc.vector.bn_aggr(mv, st6[:])

            def newton_rstd(mvv, y, w):
                vv = lnp.tile([128, NLN], F32, tag="vv", name="vv")
                nc.vector.tensor_scalar(
                    vv[:, 0:w], mvv, EPS, -0.5, op0=OP.add, op1=OP.mult,
                )   # -(var+eps)/2
                nc.vector.memset(y, 1.0)
                for _ in range(3):
                    t = lnp.tile([128, NLN], F32, tag="nt", name="nt")
                    nc.vector.tensor_tensor(t[:, 0:w], y, y, op=OP.mult)
                    nc.vector.tensor_tensor(
                        t[:, 0:w], t[:, 0:w], vv[:, 0:w], op=OP.mult
                    )
                    nc.vector.tensor_scalar(
                        t[:, 0:w], t[:, 0:w], 1.5, None, op0=OP.add
                    )
                    nc.vector.tensor_tensor(y, y, t[:, 0:w], op=OP.mult)

            def make_lhs(i, mv, rstd):
                """z -> zT (dma xbar) -> proj -> fp8 lhs rows [0:16]."""
                z = lnp.tile([128, D], BF16, tag="z")
                nc.vector.tensor_scalar(
                    z[:], xin(i), mv, rstd, op0=OP.subtract, op1=OP.mult,
                )
                zt = ztp.tile([128, 2, 128], BF16, tag="zt")
                for h in range(2):
                    nc.sync.dma_start(zt[:, h, :], z[:, 128 * h:128 * h + 128],
                                      transpose=True)
                ppj = p17.tile([128, BLK], F32, tag="s", name="ppj")
                for dc in range(2):
                    nc.tensor.matmul(
                        ppj[0:E, 0:128], pp[:, dc, :], zt[:, dc, :],
                        start=(dc == 0), stop=(dc == 1),
                    )
                lb = lhsbufs[i]
                nc.scalar.activation(
                    lb[0:16, :], ppj[0:16, 0:128], AF.Copy, scale=S_LH,
                )
                return lb

            # ---------------- score blocks ----------------
            def do_block(lb, g, ov):
                pt17 = p17.tile([128, BLK], F32, tag="s", name="pt17")
                pt18 = p18.tile([128, BLK], F32, tag="d", name="pt18")
                for h in range(2):
                    cs = slice(g * BLK + 512 * h, g * BLK + 512 * (h + 1))
                    nc.tensor.matmul(
                        pt17[:, 512 * h:512 * (h + 1)], lb[0:17, :],
                        em[0:17, cs], start=True, stop=True,
                    )
                for h in range(2):
                    cs = slice(g * BLK + 512 * h, g * BLK + 512 * (h + 1))
                    nc.tensor.matmul(
                        pt18[:, 512 * h:512 * (h + 1)], lb[0:18, :],
                        em[0:18, cs], start=True, stop=True,
                    )
                nc.vector.tensor_reduce(
                    ov[:, g:g + 1], pt17[:], axis=AX, op=OP.max, negate=True
                )
                nc.scalar.activation(
                    etr[:], pt18[:], AF.Exp, bias=ov[:, g:g + 1],
                    accum_out=ov[:, 8 + g:9 + g],
                )

            # ---------------- emission schedule ----------------
            ln_stats(0, mv0[:])
            newton_rstd(mv0[:, 1:2], rstd0[:, 0:1], 1)
            lhs = {0: make_lhs(0, mv0[:, 0:1], rstd0[:, 0:1])}

            # remaining input DMAs after the tile-0 critical chain
            if NT > 1:
                nc.scalar.dma_start(xall[:], xsm[:, 1:, :])
            if use_tail:
                nc.scalar.dma_start(xtail[:], xst[:, :])

            for i in range(1, NLN):
                ln_stats(i, mvall[:, 2 * i:2 * i + 2])
            if NLN > 1:
                newton_rstd(mvall[:, 3:2 * NLN:2], rstd_all[:, 1:NLN],
                            NLN - 1)

            def mvr(i):
                return (mvall[:, 2 * i:2 * i + 1], rstd_all[:, i:i + 1])

            def emit_tail():
                ovt = ovp.tile([128, 16], F32, tag="ov")
                do_block(lhs[NLN - 1], 0, ovt)
                nc.sync.dma_start(out[:, NT * 16:NT * 16 + 1], ovt[:, 0:1])
                nc.sync.dma_start(out[:, NT * 16 + 1:NT * 16 + 2],
                                  ovt[:, 8:9])

            tail_done = False
            for i in range(NT):
                ov = ovp.tile([128, 16], F32, tag="ov")
                for g in range(NBLK):
                    do_block(lhs[i], g, ov)
                    # lhs 1 and 2 built inside tile 0's slack; the ppj
                    # matmuls are tiny and no longer block tile 0's MMs
                    if i == 0 and g == 2 and NLN > 1 and 1 not in lhs:
                        lhs[1] = make_lhs(1, *mvr(1))
                    if i == 0 and g == 5 and NLN > 2 and 2 not in lhs:
                        lhs[2] = make_lhs(2, *mvr(2))
                    # tail runs mid-tile-2 so it doesn't serialize the drain
                    if use_tail and i == 2 and g == 4 and NLN - 1 in lhs:
                        emit_tail()
                        tail_done = True
                if i + 3 < NLN:
                    lhs[i + 3] = make_lhs(i + 3, *mvr(i + 3))
                nc.sync.dma_start(out[:, 16 * i:16 * (i + 1)], ov[:])
            if use_tail and not tail_done:
                emit_tail()

    nc.finalize()
    return nc


def _f8(x):
    import ml_dtypes

    return np.clip(np.asarray(x, np.float64), -240.0, 240.0).astype(
        ml_dtypes.float8_e4m3
    )


def _prep_in_maps(xs, pad_mask, masked_masks, ln_gamma, ln_beta, projection,
                  embeddings, top_n_out, mask_emb):
    import ml_dtypes

    xsf = np.ascontiguousarray(np.asarray(xs, np.float32).reshape(B * T, D))
    pmf = np.asarray(pad_mask).reshape(-1).astype(bool)
    mmf = np.asarray(masked_masks).reshape(-1).astype(bool)
    gamma = np.asarray(ln_gamma, np.float64)
    beta = np.asarray(ln_beta, np.float64)
    proj = np.asarray(projection, np.float64)
    emb = np.asarray(embeddings, np.float64)[0]          # [E, N]
    wmat = np.asarray(top_n_out, np.float64)[0]          # [D, N]
    maske = np.asarray(mask_emb, np.float64)

    sel = np.nonzero(pmf & mmf)[0]
    n = len(sel)
    NT = max(1, -(-max(n - 128, 1) // (NCORES * 128)))
    nmain = min(n, NCORES * 128 * NT)
    L = n - nmain
    assert L <= 128, f"tail overflow: {L}"
    use_tail = L > 0

    main_idx = sel[:nmain]
    xs_cores, m_cores = [], []
    for c in range(NCORES):
        idx = main_idx[c * 128 * NT:(c + 1) * 128 * NT]
        k = len(idx)
        xc = np.zeros((NT * 128, D), np.float32)
        mc = np.zeros((NT * 128,), np.float32)
        if k:
            xc[:k] = xsf[idx]
            mc[:k] = 1.0
        xs_cores.append(
            np.ascontiguousarray(xc.reshape(NT, 128, D).transpose(1, 0, 2))
        )
        m_cores.append(
            np.ascontiguousarray(mc.reshape(NT, 128).transpose(1, 0))
        )

    xt = np.zeros((128, D), np.float32)
    if use_tail:
        xt[:L] = xsf[sel[nmain:]]

    # host weight prep: gamma-folded projection, qrow (incl. beta fold),
    # L0 row and S0 (all frozen-weight-only quantities)
    ppf = (gamma[:, None] * proj)                        # [D, E]
    b0v = beta @ proj                                    # [E]
    L0 = maske @ wmat                                    # [N]
    qrow = -0.5 * np.sum(emb ** 2, axis=0) + b0v @ emb   # [N]
    m0 = L0.max()
    s0 = float(m0 + np.log(np.exp(L0 - m0).sum()))

    ppb = np.zeros((128, 2, E), np.float64)
    for dc in range(2):
        ppb[:, dc, :] = ppf[dc * 128:(dc + 1) * 128, :]
    ppb = ppb.astype(ml_dtypes.bfloat16)

    lhcv = np.zeros((2, 128), np.float64)
    lhcv[0, :] = C_Q
    lhcv[1, :] = C_L
    lhc = _f8(lhcv)

    in_maps = []
    for c in range(NCORES):
        perm = np.roll(np.arange(N), -c * BLK)
        em9h = np.zeros((18, N), np.float64)
        em9h[0:16, :] = S_EM * emb[:, perm]
        em9h[16, :] = S_Q * qrow[perm]
        em9h[17, :] = S_L * L0[perm]
        in_maps.append({
            "xsm": xs_cores[c],
            "xst": xt,
            "em9": _f8(em9h),
            "ppb": ppb,
            "lhc": lhc,
        })
    return in_maps, (NT, use_tail, n, L, m_cores, s0)


def kernel(**inputs) -> np.ndarray:
    in_maps, (NT, use_tail, n, L, m_cores, s0) = _prep_in_maps(**inputs)
    key = (NT, use_tail)
    if key not in _CACHE:
        _CACHE[key] = _build_bass(NT, use_tail)
        _CACHE["nc"] = _CACHE[key]
    nc = _CACHE[key]
    res = bass_utils.run_bass_kernel_spmd(
        nc, in_maps, core_ids=list(range(NCORES))
    )
    num = 0.0
    bmt = np.zeros((NCORES, 128), np.float64)
    vst = np.zeros((NCORES, 128), np.float64)
    for c, r in enumerate(res.results):
        o = np.asarray(r["out"], np.float64).reshape(128, NT * 16 + 2)
        m = m_cores[c].astype(np.float64)                 # [128, NT]
        for t in range(NT):
            nb = o[:, 16 * t:16 * t + 8]
            vs = o[:, 16 * t + 8:16 * t + 16]
            m1 = (-nb).max(axis=1)
            vtot = (vs * np.exp(-nb - m1[:, None])).sum(axis=1)
            lv = np.log(np.maximum(vtot, 1e-300))   # pad rows may be junk
            num += float((m[:, t] * lv).sum())
        bmt[c] = -o[:, NT * 16]
        vst[c] = o[:, NT * 16 + 1]
    if L > 0:
        gm = bmt.max(axis=0)
        w = np.exp(bmt - gm[None, :])
        vtot = (vst * w).sum(axis=0)
        num += float(np.log(np.maximum(vtot[:L], 1e-300)).sum())
    loss = np.float32(s0 - num / BD / n)
    return np.asarray(loss, np.float32)


# revision 29
# speedup vs baseline: 1.0195x; 1.0195x over previous
"""BestRQ loss kernel for 8 Trainium2 NeuronCores (v5).

Math (exact reformulations of the reference):
  - Only masked tokens contribute; their logits row is shared:
    loss = S0 - mean_t L0[target_t], L0 = mask_emb @ W, S0 = logsumexp(L0).
  - target_t = argmax_n score_tn, score_tn = proj_t.emb_n - 0.5|emb_n|^2.
  - S0, L0, qrow (= -0.5|emb|^2 + (beta@P).emb) depend only on frozen
    weights -> host prep (same class as qrow/perm prep in the baseline).
  - Device per 1024-code block: pt17 = beta*score (fp8 matmul, K=17),
    pt18 = beta*score + 20*L0 (K=18, same stationary tile, L0 row last).
    nb = -max(pt17) on DVE; vsum = sum exp(pt18+nb) on ACT.  The winner
    term is exp(20*L0[argmax]); the host combines blocks/cores with a
    logsumexp and divides by 20.
  - beta is folded into the fp8 operand scales (emb x32, lhs x32, qrow x8
    coef 128, L0 x64 coef 0.3125 -> eff beta = 1024, eff beta*delta = 20.0
    exactly; TRN fp8e4 is OCP e4m3, max normal 240).  fp8 per-element
    noise only perturbs near-ties, which is L0-unbiased.
  - 4096 masked tokens -> 4 tiles x 128 per core; <=128 leftovers are
    replicated on every core as a tail tile where each core scores its own
    1/8 of the (block-rotated) codebook; host combines the partials.
"""

import math

import numpy as np

try:
    import concourse.bass as bass  # noqa: F401
except ImportError:  # pragma: no cover
    import sys

    sys.path.insert(0, "/opt/trn_rl_repo")
    import concourse.bass as bass  # noqa: F401

import concourse.mybir as mybir
from concourse import bacc, bass_utils
from concourse.tile import TileContext

F32 = mybir.dt.float32
BF16 = mybir.dt.bfloat16
FP8 = mybir.dt.float8e4

B, T, D, E, N = 16, 512, 256, 16, 8192
NCORES = 8
EPS = 1e-5
NBLK = 8
BLK = N // NBLK

# TRN fp8e4 is OCP e4m3 (max normal 240, has inf) -> keep operands < 240
S_EM = 32.0     # emb row scale
S_LH = 32.0     # proj lhs scale (eff beta = S_EM*S_LH = 1024)
S_Q = 8.0       # qrow row scale
C_Q = 128.0     # qrow lhs coef (C_Q*S_Q = 1024)
S_L = 64.0      # L0 row scale
C_L = 0.3125    # L0 lhs coef -> eff beta*delta = S_L*C_L = 20.0 (exact fp8)
BD = S_L * C_L  # 20.0

_CACHE = {}


def _build_bass(NT, use_tail):
    nc = bacc.Bacc(
        "TRN2", target_bir_lowering=False, debug=False, num_devices=NCORES
    )
    NLN = NT + (1 if use_tail else 0)
    NOUT = NT * 16 + 2
    xsm = nc.dram_tensor("xsm", [128, NT, D], F32, kind="ExternalInput")
    xst = nc.dram_tensor("xst", [128, D], F32, kind="ExternalInput")
    em9 = nc.dram_tensor("em9", [18, N], FP8, kind="ExternalInput")
    ppb = nc.dram_tensor("ppb", [128, 2, E], BF16, kind="ExternalInput")
    lhc = nc.dram_tensor("lhc", [2, 128], FP8, kind="ExternalInput")
    idin = nc.dram_tensor("idin", [128, 128], BF16, kind="ExternalInput")
    out = nc.dram_tensor("out", [128, NOUT], F32, kind="ExternalOutput")

    AX = mybir.AxisListType.X
    OP = mybir.AluOpType
    AF = mybir.ActivationFunctionType

    with TileContext(nc) as tc:
        with (
            tc.tile_pool(name="cst", bufs=1) as cst,
            tc.tile_pool(name="lnp", bufs=2) as lnp,
            tc.tile_pool(name="ztp", bufs=2) as ztp,
            tc.tile_pool(name="lhp", bufs=1) as lhp,
            tc.tile_pool(name="ovp", bufs=2) as ovp,
            tc.tile_pool(name="p17", bufs=2, space="PSUM") as p17,
            tc.tile_pool(name="p18", bufs=2, space="PSUM") as p18,
        ):
            # ---------------- DMAs (SP + ACT hwdge queues) ----------------
            x0t = cst.tile([128, D], F32)
            xall = cst.tile([128, max(NT - 1, 1), D], F32)
            xtail = cst.tile([128, D], F32)
            em = cst.tile([18, N], FP8)
            pp = cst.tile([128, 2, E], BF16)

            ident = cst.tile([128, 128], BF16)
            nc.sync.dma_start(x0t[:], xsm[:, 0, :])
            for q in range(4):
                cs = slice(q * (N // 4), (q + 1) * (N // 4))
                nc.sync.dma_start(em[:, cs], em9[:, cs])
            nc.scalar.dma_start(pp[:], ppb[:, :, :])
            nc.scalar.dma_start(ident[:], idin[:, :])
            lhsbufs = []
            for i in range(NLN):
                lb = lhp.tile([18, 128], FP8, tag="lhs", name="lhs",
                              bufs=NLN)
                nc.scalar.dma_start(lb[16:18, :], lhc[:, :])
                lhsbufs.append(lb)

            # preload the Exp table while DMAs are in flight
            wz = cst.tile([1, 1], F32)
            nc.vector.memset(wz[:], 0.0)
            we = cst.tile([1, 1], F32)
            nc.scalar.activation(we[0:1, :], wz[0:1, :], AF.Exp)

            mv0 = cst.tile([128, 2], F32)
            rstd0 = cst.tile([128, 1], F32)
            mvall = cst.tile([128, 2 * NLN], F32)
            rstd_all = cst.tile([128, NLN], F32)
            etr = cst.tile([128, BLK], BF16)   # exp trash output

            def xin(i):
                if i == 0:
                    return x0t[:]
                return xall[:, i - 1, :] if i < NT else xtail[:]

            # ------------- LN stats; rsqrt via Newton (no tables) ---------
            def ln_stats(i, mv):
                st6 = lnp.tile([128, 6], F32, tag="st6")
                nc.vector.bn_stats(st6[:], xin(i))
                nc.vector.bn_aggr(mv, st6[:])

            def newton_rstd(mvv, y, w):
                vv = lnp.tile([128, NLN], F32, tag="vv", name="vv")
                nc.vector.tensor_scalar(
                    vv[:, 0:w], mvv, EPS, -0.5, op0=OP.add, op1=OP.mult,
                )   # -(var+eps)/2
                nc.vector.memset(y, 1.0)
                for _ in range(3):
                    t = lnp.tile([128, NLN], F32, tag="nt", name="nt")
                    nc.vector.tensor_tensor(t[:, 0:w], y, y, op=OP.mult)
                    nc.vector.tensor_tensor(
                        t[:, 0:w], t[:, 0:w], vv[:, 0:w], op=OP.mult
                    )
                    nc.vector.tensor_scalar(
                        t[:, 0:w], t[:, 0:w], 1.5, None, op0=OP.add
                    )
                    nc.vector.tensor_tensor(y, y, t[:, 0:w], op=OP.mult)

            def make_lhs(i, mv, rstd):
                """z -> zT (PE transpose) -> proj -> fp8 lhs rows [0:16]."""
                z = lnp.tile([128, D], BF16, tag="z")
                nc.vector.tensor_scalar(
                    z[:], xin(i), mv, rstd, op0=OP.subtract, op1=OP.mult,
                )
                mtz = p18.tile([128, BLK], F32, tag="d", name="mtz")
                ztb = mtz[:].bitcast(BF16)[:, 0:256]
                for h in range(2):
                    nc.tensor.transpose(
                        ztb[:, h * 128:(h + 1) * 128],
                        z[:, h * 128:(h + 1) * 128], ident[:],
                    )
                zt = ztp.tile([128, 2, 128], BF16, tag="zt")
                nc.scalar.activation(zt[:, 0, :], ztb[:, 0:128], AF.Copy)
                nc.scalar.activation(zt[:, 1, :], ztb[:, 128:256], AF.Copy)
                ppj = p17.tile([128, BLK], F32, tag="s", name="ppj")
                for dc in range(2):
                    nc.tensor.matmul(
                        ppj[0:E, 0:128], pp[:, dc, :], zt[:, dc, :],
                        start=(dc == 0), stop=(dc == 1),
                    )
                lb = lhsbufs[i]
                nc.scalar.activation(
                    lb[0:16, :], ppj[0:16, 0:128], AF.Copy, scale=S_LH,
                )
                return lb

            # ---------------- score blocks ----------------
            def do_block(lb, g, ov):
                pt17 = p17.tile([128, BLK], F32, tag="s", name="pt17")
                pt18 = p18.tile([128, BLK], F32, tag="d", name="pt18")
                for h in range(2):
                    cs = slice(g * BLK + 512 * h, g * BLK + 512 * (h + 1))
                    nc.tensor.matmul(
                        pt17[:, 512 * h:512 * (h + 1)], lb[0:17, :],
                        em[0:17, cs], start=True, stop=True,
                    )
                for h in range(2):
                    cs = slice(g * BLK + 512 * h, g * BLK + 512 * (h + 1))
                    nc.tensor.matmul(
                        pt18[:, 512 * h:512 * (h + 1)], lb[0:18, :],
                        em[0:18, cs], start=True, stop=True,
                    )
                nc.vector.tensor_reduce(
                    ov[:, g:g + 1], pt17[:], axis=AX, op=OP.max, negate=True
                )
                nc.scalar.activation(
                    etr[:], pt18[:], AF.Exp, bias=ov[:, g:g + 1],
                    accum_out=ov[:, 8 + g:9 + g],
                )

            # ---------------- emission schedule ----------------
            ln_stats(0, mv0[:])
            newton_rstd(mv0[:, 1:2], rstd0[:, 0:1], 1)
            lhs = {0: make_lhs(0, mv0[:, 0:1], rstd0[:, 0:1])}

            # remaining input DMAs after the tile-0 critical chain
            if NT > 1:
                nc.scalar.dma_start(xall[:], xsm[:, 1:, :])
            if use_tail:
                nc.scalar.dma_start(xtail[:], xst[:, :])

            for i in range(1, NLN):
                ln_stats(i, mvall[:, 2 * i:2 * i + 2])
            if NLN > 1:
                newton_rstd(mvall[:, 3:2 * NLN:2], rstd_all[:, 1:NLN],
                            NLN - 1)

            def mvr(i):
                return (mvall[:, 2 * i:2 * i + 1], rstd_all[:, i:i + 1])

            def emit_tail():
                ovt = ovp.tile([128, 16], F32, tag="ov")
                do_block(lhs[NLN - 1], 0, ovt)
                nc.sync.dma_start(out[:, NT * 16:NT * 16 + 1], ovt[:, 0:1])
                nc.sync.dma_start(out[:, NT * 16 + 1:NT * 16 + 2],
                                  ovt[:, 8:9])

            tail_done = False
            for i in range(NT):
                ov = ovp.tile([128, 16], F32, tag="ov")
                for g in range(NBLK):
                    do_block(lhs[i], g, ov)
                    # lhs 1 and 2 built inside tile 0's slack; the ppj
                    # matmuls are tiny and no longer block tile 0's MMs
                    if i == 0 and g == 2 and NLN > 1 and 1 not in lhs:
                        lhs[1] = make_lhs(1, *mvr(1))
                    if i == 0 and g == 5 and NLN > 2 and 2 not in lhs:
                        lhs[2] = make_lhs(2, *mvr(2))
                    # tail runs mid-tile-2 so it doesn't serialize the drain
                    if use_tail and i == 2 and g == 4 and NLN - 1 in lhs:
                        emit_tail()
                        tail_done = True
                if i + 3 < NLN:
                    lhs[i + 3] = make_lhs(i + 3, *mvr(i + 3))
                nc.sync.dma_start(out[:, 16 * i:16 * (i + 1)], ov[:])
            if use_tail and not tail_done:
                emit_tail()

    nc.finalize()
    return nc


def _f8(x):
    import ml_dtypes

    return np.clip(np.asarray(x, np.float64), -240.0, 240.0).astype(
        ml_dtypes.float8_e4m3
    )


def _prep_in_maps(xs, pad_mask, masked_masks, ln_gamma, ln_beta, projection,
                  embeddings, top_n_out, mask_emb):
    import ml_dtypes

    xsf = np.ascontiguousarray(np.asarray(xs, np.float32).reshape(B * T, D))
    pmf = np.asarray(pad_mask).reshape(-1).astype(bool)
    mmf = np.asarray(masked_masks).reshape(-1).astype(bool)
    gamma = np.asarray(ln_gamma, np.float64)
    beta = np.asarray(ln_beta, np.float64)
    proj = np.asarray(projection, np.float64)
    emb = np.asarray(embeddings, np.float64)[0]          # [E, N]
    wmat = np.asarray(top_n_out, np.float64)[0]          # [D, N]
    maske = np.asarray(mask_emb, np.float64)

    sel = np.nonzero(pmf & mmf)[0]
    n = len(sel)
    NT = max(1, -(-max(n - 128, 1) // (NCORES * 128)))
    nmain = min(n, NCORES * 128 * NT)
    L = n - nmain
    assert L <= 128, f"tail overflow: {L}"
    use_tail = L > 0

    main_idx = sel[:nmain]
    xs_cores, m_cores = [], []
    for c in range(NCORES):
        idx = main_idx[c * 128 * NT:(c + 1) * 128 * NT]
        k = len(idx)
        xc = np.zeros((NT * 128, D), np.float32)
        mc = np.zeros((NT * 128,), np.float32)
        if k:
            xc[:k] = xsf[idx]
            mc[:k] = 1.0
        xs_cores.append(
            np.ascontiguousarray(xc.reshape(NT, 128, D).transpose(1, 0, 2))
        )
        m_cores.append(
            np.ascontiguousarray(mc.reshape(NT, 128).transpose(1, 0))
        )

    xt = np.zeros((128, D), np.float32)
    if use_tail:
        xt[:L] = xsf[sel[nmain:]]

    # host weight prep: gamma-folded projection, qrow (incl. beta fold),
    # L0 row and S0 (all frozen-weight-only quantities)
    ppf = (gamma[:, None] * proj)                        # [D, E]
    b0v = beta @ proj                                    # [E]
    L0 = maske @ wmat                                    # [N]
    qrow = -0.5 * np.sum(emb ** 2, axis=0) + b0v @ emb   # [N]
    m0 = L0.max()
    s0 = float(m0 + np.log(np.exp(L0 - m0).sum()))

    ppb = np.zeros((128, 2, E), np.float64)
    for dc in range(2):
        ppb[:, dc, :] = ppf[dc * 128:(dc + 1) * 128, :]
    ppb = ppb.astype(ml_dtypes.bfloat16)

    lhcv = np.zeros((2, 128), np.float64)
    lhcv[0, :] = C_Q
    lhcv[1, :] = C_L
    lhc = _f8(lhcv)

    in_maps = []
    for c in range(NCORES):
        perm = np.roll(np.arange(N), -c * BLK)
        em9h = np.zeros((18, N), np.float64)
        em9h[0:16, :] = S_EM * emb[:, perm]
        em9h[16, :] = S_Q * qrow[perm]
        em9h[17, :] = S_L * L0[perm]
        in_maps.append({
            "xsm": xs_cores[c],
            "xst": xt,
            "em9": _f8(em9h),
            "ppb": ppb,
            "lhc": lhc,
            "idin": np.eye(128, dtype=np.float32).astype(ml_dtypes.bfloat16),
        })
    return in_maps, (NT, use_tail, n, L, m_cores, s0)


def kernel(**inputs) -> np.ndarray:
    in_maps, (NT, use_tail, n, L, m_cores, s0) = _prep_in_maps(**inputs)
    key = (NT, use_tail)
    if key not in _CACHE:
        _CACHE[key] = _build_bass(NT, use_tail)
        _CACHE["nc"] = _CACHE[key]
    nc = _CACHE[key]
    res = bass_utils.run_bass_kernel_spmd(
        nc, in_maps, core_ids=list(range(NCORES))
    )
    num = 0.0
    bmt = np.zeros((NCORES, 128), np.float64)
    vst = np.zeros((NCORES, 128), np.float64)
    for c, r in enumerate(res.results):
        o = np.asarray(r["out"], np.float64).reshape(128, NT * 16 + 2)
        m = m_cores[c].astype(np.float64)                 # [128, NT]
        for t in range(NT):
            nb = o[:, 16 * t:16 * t + 8]
            vs = o[:, 16 * t + 8:16 * t + 16]
            m1 = (-nb).max(axis=1)
            vtot = (vs * np.exp(-nb - m1[:, None])).sum(axis=1)
            lv = np.log(np.maximum(vtot, 1e-300))   # pad rows may be junk
            num += float((m[:, t] * lv).sum())
        bmt[c] = -o[:, NT * 16]
        vst[c] = o[:, NT * 16 + 1]
    if L > 0:
        gm = bmt.max(axis=0)
        w = np.exp(bmt - gm[None, :])
        vtot = (vst * w).sum(axis=0)
        num += float(np.log(np.maximum(vtot[:L], 1e-300)).sum())
    loss = np.float32(s0 - num / BD / n)
    return np.asarray(loss, np.float32)
